# revision 15
# baseline (speedup 1.0000x reference)
"""BotRGCN (2-layer RGCN + encoders + classifier) on 8 Trainium2 NeuronCores.

Strategy
--------
Nodes are sharded across the 8 cores by destination id (12500 nodes/core).
Host preprocessing groups each core's in-edges into (dst-group of 256 nodes,
relation, src-bank) sub-segments, whose chunk counts are padded to a uniform
cross-core envelope so all cores run one SPMD program.

Per layer, messages h[src] are fetched with `dma_gather` (gpsimd ext-isa
kernel; int16 bank-local indices, trailing -1 padding trimmed by the ucode)
from a node-major bf16 table in DRAM.  Four SWDGE queues run four gathers
concurrently on four Q7 core pairs (~3.6 ns/row vs 8.9 single-queue).  Per
128-edge chunk, a one-hot selection matrix S[e, i] = (dst_off[e] == i) *
inv_deg is built on the vector engine with one tensor_scalar(is_equal, mult)
in the 4x DVE mode, and the tensor engine accumulates
agg_t[d, i] += G[e, d]^T S[e, i] into PSUM (feature-major, 256 columns).

All node-feature data on chip is feature-major [128 feat, nodes], so weight
matmuls are `matmul(lhsT=W, rhs=h_t)` and biases fuse into PSUM->SBUF copies
on the scalar engine; LeakyReLU/PReLU are computed as max(a*x, x) with one
DVE scalar_tensor_tensor.  Between layers each core PE-transposes its
h-shard to node-major, writes a DRAM bounce buffer, and an AllGather
rebuilds the full table on every core.

Gather buffers are persistent and memset once: slots skipped by the
trailing-trim keep stale SBUF bytes on HW, so buffers must never hold
non-finite bits; dead slots have S rows = 0, so finite residue contributes
exactly zero.  The per-core valid-index count is computed on-device as
(num_idxs - pad_count[input]) so the Tile scheduling sim (which runs with
zeroed inputs) and the functional sim both see a consistent value; the HW
ucode ignores the register and trims trailing -1s itself.
"""

import os
import sys

import numpy as np


def _ensure_path():
    try:
        import concourse  # noqa: F401
    except ImportError:
        for p in ("/opt/trn_rl_repo",):
            if p not in sys.path and os.path.isdir(p):
                sys.path.insert(0, p)


_ensure_path()

import ml_dtypes  # noqa: E402

BF16 = ml_dtypes.bfloat16

# ---------------------------------------------------------------- constants
N_NODES = 100000
N_EDGES = 1600000
NUM_PROP = 5
CAT_PROP = 3
D = 128
NCORES = 8
GROUP = 256            # dst nodes per aggregation group (S width)
NREL = 2
BANK = 25000           # src-bank rows (int16 indices must stay < 32768)
NQUEUE = 4
NGBUF = int(os.environ.get("KERNEL_NGBUF", "4"))


# ============================================================ preprocessing
def preprocess(edge_index, edge_type, n_nodes=N_NODES, ncores=NCORES,
               group=GROUP, bank=BANK):
    """Pack edges into per-core dma_gather index arrays + uniform schedule.

    Edge slots are ordered by (dst-group, rel, src-bank); each (g, r, b)
    sub-segment gets an envelope chunk count (max over cores); each core
    fills its slots contiguously and pads the tail with -1 (not gathered).

    Returns dict with:
      gidx: [ncores, 16, tot16] int16, 16-row-wrapped dma_gather indices
            (bank-local; slot j of a call -> [j%16, coff16 + j//16])
      gdst: [ncores, 128, totch] f32, dst offset in group per (lane, chunk)
            (sentinel 300 -> S row zero)
      gscl: [ncores, 128, totch] f32, 1/deg of dst (0 for dead slots)
      gpad: [ncores, nseg] int32, per-call pad counts (num_idxs - valid)
      sched: sched[g][r][b] = (seg_id, coff16, blk0, nblk) or None;
             blk0 = chunk offset inside the (g, r) gather buffer.
      chcol: (g, r, b) -> first gdst/gscl chunk column
    """
    src = np.asarray(edge_index[0]).astype(np.int64)
    dst = np.asarray(edge_index[1]).astype(np.int64)
    rel = np.asarray(edge_type).astype(np.int64)
    nsh = n_nodes // ncores
    assert nsh * ncores == n_nodes
    ngrp = -(-nsh // group)
    nbank = -(-n_nodes // bank)

    deg = np.stack(
        [np.bincount(dst[rel == r], minlength=n_nodes) for r in range(NREL)]
    )
    invdeg = (1.0 / np.maximum(deg, 1)).astype(np.float32)

    core = dst // nsh
    grp = (dst % nsh) // group
    off = (dst % nsh) % group
    bnk = src // bank

    order = np.lexsort((bnk, rel, grp, core))
    ssrc = src[order]
    soff = off[order]
    sinv = invdeg[rel[order], dst[order]]

    segkey = (((core * ngrp + grp) * NREL + rel) * nbank + bnk)[order]
    nsegs_all = ncores * ngrp * NREL * nbank
    seg_start = np.searchsorted(segkey, np.arange(nsegs_all))
    seg_end = np.searchsorted(segkey, np.arange(nsegs_all) + 1)
    cnts = (seg_end - seg_start).reshape(ncores, ngrp, NREL, nbank)
    env_chunks = -(-cnts.max(axis=0) // 128)  # [ngrp, NREL, nbank]

    sched = []
    tot16 = 0
    nseg = 0
    maxblk = 0
    totch = 0
    chcol = {}
    for g in range(ngrp):
        row = []
        for r in range(NREL):
            blk0 = 0
            banks = []
            for b in range(nbank):
                nblk = int(env_chunks[g, r, b])
                chcol[(g, r, b)] = totch
                totch += nblk
                if nblk == 0:
                    banks.append(None)
                    continue
                banks.append((nseg, tot16, blk0, nblk))
                nseg += 1
                tot16 += nblk * 8
                blk0 += nblk
            row.append(banks)
            maxblk = max(maxblk, blk0)
        sched.append(row)

    gidx = np.full((ncores, 16, max(tot16, 1)), -1, np.int16)
    gdst = np.full((ncores, 128, max(totch, 1)), 300.0, np.float32)
    gscl = np.zeros((ncores, 128, max(totch, 1)), np.float32)
    gpad = np.zeros((ncores, max(nseg, 1)), np.int32)

    for g in range(ngrp):
        for r in range(NREL):
            for b in range(nbank):
                ent = sched[g][r][b]
                if ent is None:
                    continue
                seg_id, coff16, blk0, nblk = ent
                num = nblk * 128
                cc0 = chcol[(g, r, b)]
                for c in range(ncores):
                    s = seg_start[((c * ngrp + g) * NREL + r) * nbank + b]
                    e = seg_end[((c * ngrp + g) * NREL + r) * nbank + b]
                    cnt = e - s
                    if cnt == 0:
                        # interp requires >= 1 valid index per call
                        gidx[c, 0, coff16] = 0
                        gpad[c, seg_id] = num - 1
                        continue
                    j = np.arange(cnt)
                    vals = (ssrc[s:e] - b * bank).astype(np.int16)
                    gidx[c, j % 16, coff16 + j // 16] = vals
                    gpad[c, seg_id] = num - cnt
                    gdst[c, j % 128, cc0 + j // 128] = soff[s:e]
                    gscl[c, j % 128, cc0 + j // 128] = sinv[s:e]
    return dict(gidx=gidx, gdst=gdst, gscl=gscl, gpad=gpad, sched=sched,
                chcol=chcol, totch=max(totch, 1), tot16=max(tot16, 1),
                nseg=nseg, ngrp=ngrp, nsh=nsh, maxblk=maxblk, nbank=nbank,
                bank=bank)


# =============================================================== host model
def host_model(inputs, pack, n_nodes=N_NODES, ncores=NCORES, group=GROUP):
    """Numpy emulation of the device algorithm (bf16 rounding included)."""
    def bf(x):
        return np.asarray(x, np.float32).astype(BF16).astype(np.float32)

    x = np.asarray(inputs["x"], np.float32)
    nsh = n_nodes // ncores

    def leaky(v):
        return np.where(v >= 0, v, 0.01 * v)

    n = leaky(x[:, :NUM_PROP] @ np.asarray(inputs["W_num"], np.float32)
              + inputs["b_num"])
    c = leaky(x[:, NUM_PROP:] @ np.asarray(inputs["W_cat"], np.float32)
              + inputs["b_cat"])
    h = np.concatenate([n, c], axis=1)
    h = h @ np.asarray(inputs["W_in"], np.float32) + inputs["b_in"]
    h0 = bf(np.where(h >= 0, h, np.asarray(inputs["prelu_a"]) * h))

    gidx, gdst, gscl, gpad = (pack["gidx"], pack["gdst"], pack["gscl"],
                              pack["gpad"])
    sched, chcol = pack["sched"], pack["chcol"]
    ngrp, nbank, bank = pack["ngrp"], pack["nbank"], pack["bank"]

    def rgcn(table, W_root, W_rel, bias, h_own):
        out = np.zeros((n_nodes, D), np.float32)
        for cc in range(ncores):
            for g in range(ngrp):
                g0 = g * group
                gw = min(group, nsh - g0)
                hn = (bf(h_own[cc * nsh + g0: cc * nsh + g0 + gw])
                      @ bf(W_root).astype(np.float32))
                for r in range(NREL):
                    agg = np.zeros((group, D), np.float32)
                    for b in range(nbank):
                        ent = sched[g][r][b]
                        if ent is None:
                            continue
                        seg_id, coff16, blk0, nblk = ent
                        cc0 = chcol[(g, r, b)]
                        num = nblk * 128
                        valid = num - gpad[cc, seg_id]
                        jj = np.arange(valid)
                        idx = gidx[cc, jj % 16, coff16 + jj // 16].astype(
                            np.int64)
                        G = np.zeros((num, D), np.float32)
                        G[:valid] = bf(table[idx + b * bank])
                        for k in range(nblk):
                            dstv = gdst[cc, :, cc0 + k]
                            sclv = gscl[cc, :, cc0 + k]
                            S = (dstv[:, None] ==
                                 np.arange(group)[None, :]).astype(np.float32)
                            S = bf(S * sclv[:, None])
                            agg += S.T @ G[k * 128:(k + 1) * 128]
                    hn = hn + bf(agg[:gw]) @ bf(W_rel[r]).astype(np.float32)
                out[cc * nsh + g0: cc * nsh + g0 + gw] = hn + bias
        return out

    h1 = rgcn(bf(h0), inputs["W_root1"], inputs["W_rel1"],
              inputs["b_rgcn1"], h0)
    h1b = bf(h1)
    h2 = rgcn(h1b, inputs["W_root2"], inputs["W_rel2"], inputs["b_rgcn2"],
              h1b)
    h2b = bf(h2)
    return h2b @ bf(np.asarray(inputs["W_cls"], np.float32)) + inputs["b_cls"]


# ============================================================ device program
def build_program(pack, n_nodes=N_NODES, ncores=NCORES, group=GROUP,
                  enable_asserts=False, debug_outputs=False):
    _ensure_path()
    import contextlib

    import concourse.bacc as bacc
    import concourse.mybir as mybir
    import concourse.tile as tile

    dt = mybir.dt
    Alu = mybir.AluOpType
    Act = mybir.ActivationFunctionType

    nsh = pack["nsh"]
    ngrp = pack["ngrp"]
    totch = pack["totch"]
    tot16 = pack["tot16"]
    nseg = pack["nseg"]
    maxblk = pack["maxblk"]
    nbank = pack["nbank"]
    bank = pack["bank"]
    sched = pack["sched"]
    chcol = pack["chcol"]
    ntile = -(-nsh // 512)

    nc = bacc.Bacc("TRN2", target_bir_lowering=False, debug=False,
                   enable_asserts=enable_asserts, num_devices=ncores,
                   num_swdge_queues=NQUEUE)

    def din(name, shape, dtype):
        return nc.dram_tensor(name, shape, dtype, kind="ExternalInput")

    xnum = din("xnum", [NUM_PROP, nsh], dt.float32)
    xcat = din("xcat", [CAT_PROP, nsh], dt.float32)
    w_num = din("w_num", [NUM_PROP, 64], dt.float32)
    w_cat = din("w_cat", [CAT_PROP, 64], dt.float32)
    w_in = din("w_in", [D, D], dt.float32)
    b_num = din("b_num", [64, 1], dt.float32)
    b_cat = din("b_cat", [64, 1], dt.float32)
    b_in = din("b_in", [D, 1], dt.float32)
    prelu_a = din("prelu_a", [D, 1], dt.float32)
    wl = {}
    for L in (1, 2):
        wl[L] = dict(
            root=din(f"w_root{L}", [D, D], dt.bfloat16),
            rel0=din(f"w_rel{L}0", [D, D], dt.bfloat16),
            rel1=din(f"w_rel{L}1", [D, D], dt.bfloat16),
            bias=din(f"b_rgcn{L}", [D, 1], dt.float32),
        )
    w_cls = din("w_cls", [D, D], dt.bfloat16)
    b_cls = din("b_cls", [D, 1], dt.float32)
    gidx_d = din("gidx", [128, tot16], dt.int16)
    gdst_d = din("gdst", [128, totch], dt.float32)
    gscl_d = din("gscl", [128, totch], dt.float32)
    gpad_d = din("gpad", [1, max(nseg, 1)], dt.int32)
    iota_d = din("iotaG", [128, group], dt.bfloat16)
    ident_d = din("ident128", [128, 128], dt.bfloat16)

    out_t = nc.dram_tensor("out_t", [D, nsh], dt.float32,
                           kind="ExternalOutput")
    dbg = {}
    if debug_outputs:
        for nm, shp in (("d_h0", [D, nsh]), ("d_h1", [D, nsh])):
            dbg[nm] = nc.dram_tensor(nm, shp, dt.float32,
                                     kind="ExternalOutput")
        dbg["d_table0"] = nc.dram_tensor("d_table0", [n_nodes, D],
                                         dt.bfloat16,
                                         kind="ExternalOutput")

    shard = [nc.dram_tensor(f"shard{L}", [nsh, D], dt.bfloat16)
             for L in range(2)]
    table = [nc.dram_tensor(f"table{L}", [n_nodes, D], dt.bfloat16,
                            addr_space="Shared") for L in range(2)]
    rg = [list(range(ncores))]

    with tile.TileContext(nc) as tc:
        ctx = contextlib.ExitStack()
        with ctx:
            constp = ctx.enter_context(tc.tile_pool(name="const", bufs=1))
            bigp = ctx.enter_context(tc.tile_pool(name="big", bufs=1))
            workp = ctx.enter_context(tc.tile_pool(name="work", bufs=3))
            spool = ctx.enter_context(tc.tile_pool(name="spool", bufs=4))
            psC = ctx.enter_context(tc.tile_pool(name="psC", bufs=2,
                                                 space="PSUM"))

            def load_const(dram, shape, dtype):
                t = constp.tile(shape, dtype, tag=f"c_{dram.name}")
                nc.sync.dma_start(out=t[:], in_=dram[:, :])
                return t

            iota_sb = load_const(iota_d, [128, group], dt.bfloat16)
            ident_sb = load_const(ident_d, [128, 128], dt.bfloat16)
            wnum_sb = load_const(w_num, [NUM_PROP, 64], dt.float32)
            wcat_sb = load_const(w_cat, [CAT_PROP, 64], dt.float32)
            win_sb = load_const(w_in, [D, D], dt.float32)
            bnum_sb = load_const(b_num, [64, 1], dt.float32)
            bcat_sb = load_const(b_cat, [64, 1], dt.float32)
            bin_sb = load_const(b_in, [D, 1], dt.float32)
            pa_sb = load_const(prelu_a, [D, 1], dt.float32)
            wl_sb = {}
            for L in (1, 2):
                wl_sb[L] = dict(
                    root=load_const(wl[L]["root"], [D, D], dt.bfloat16),
                    rel0=load_const(wl[L]["rel0"], [D, D], dt.bfloat16),
                    rel1=load_const(wl[L]["rel1"], [D, D], dt.bfloat16),
                    bias=load_const(wl[L]["bias"], [D, 1], dt.float32),
                )
            wcls_sb = load_const(w_cls, [D, D], dt.bfloat16)
            bcls_sb = load_const(b_cls, [D, 1], dt.float32)
            gidx_sb = load_const(gidx_d, [128, tot16], dt.int16)
            gdst_sb = load_const(gdst_d, [128, totch], dt.float32)
            gscl_sb = load_const(gscl_d, [128, totch], dt.float32)
            gpad_sb = load_const(gpad_d, [1, max(nseg, 1)], dt.int32)

            h0_sb = bigp.tile([D, nsh], dt.bfloat16, tag="h0")
            h1_sb = bigp.tile([D, nsh], dt.bfloat16, tag="h1")

            gbufs = []
            for i in range(NGBUF):
                gb = bigp.tile([128, maxblk, D], dt.bfloat16, tag=f"gbuf{i}")
                gbufs.append(gb)
            for gb in gbufs:
                nc.vector.memset(gb[:], 0.0)
            state = dict(gb=0)

            regs = [nc.gpsimd.alloc_register(f"vcnt{i}")
                    for i in range(NQUEUE)]

            def transpose_to_shard(src_sb, col0, gw, shard_dram):
                for s0 in range(0, gw, 128):
                    w = min(128, gw - s0)
                    trp = psC.tile([128, 128], dt.bfloat16, tag="aux")
                    nc.tensor.transpose(
                        out=trp[:w, :],
                        in_=src_sb[:, col0 + s0:col0 + s0 + w],
                        identity=ident_sb[:])
                    st = workp.tile([128, D], dt.bfloat16, tag="st")
                    nc.scalar.activation(out=st[:w, :], in_=trp[:w, :],
                                         func=Act.Copy)
                    nc.sync.dma_start(
                        out=shard_dram[col0 + s0:col0 + s0 + w, :],
                        in_=st[:w, :])

            # ================= encoder =================
            encctx = contextlib.ExitStack()
            psE = encctx.enter_context(
                tc.tile_pool(name="psE", bufs=2, space="PSUM"))
            for t in range(ntile):
                c0 = t * 512
                w = min(512, nsh - c0)
                xn = workp.tile([NUM_PROP, 512], dt.float32, tag="xn")
                xc = workp.tile([CAT_PROP, 512], dt.float32, tag="xc")
                nc.sync.dma_start(out=xn[:, :w], in_=xnum[:, c0:c0 + w])
                nc.sync.dma_start(out=xc[:, :w], in_=xcat[:, c0:c0 + w])
                pn = psE.tile([64, 512], dt.float32, tag="pn")
                pc = psE.tile([64, 512], dt.float32, tag="pc")
                nc.tensor.matmul(out=pn[:, :w], lhsT=wnum_sb[:],
                                 rhs=xn[:, :w], start=True, stop=True)
                nc.tensor.matmul(out=pc[:, :w], lhsT=wcat_sb[:],
                                 rhs=xc[:, :w], start=True, stop=True)
                tmp = workp.tile([D, 512], dt.float32, tag="enc_tmp")
                nc.scalar.activation(out=tmp[0:64, :w], in_=pn[:, :w],
                                     func=Act.Identity, bias=bnum_sb[:])
                nc.scalar.activation(out=tmp[64:128, :w], in_=pc[:, :w],
                                     func=Act.Identity, bias=bcat_sb[:])
                henc = workp.tile([D, 512], dt.float32, tag="henc")
                nc.vector.scalar_tensor_tensor(
                    out=henc[:, :w], in0=tmp[:, :w], scalar=0.01,
                    in1=tmp[:, :w], op0=Alu.mult, op1=Alu.max)
                ph = psE.tile([D, 512], dt.float32, tag="ph")
                nc.tensor.matmul(out=ph[:, :w], lhsT=win_sb[:],
                                 rhs=henc[:, :w], start=True, stop=True)
                tmp2 = workp.tile([D, 512], dt.float32, tag="enc_tmp2")
                nc.scalar.activation(out=tmp2[:, :w], in_=ph[:, :w],
                                     func=Act.Identity, bias=bin_sb[:])
                nc.vector.scalar_tensor_tensor(
                    out=h0_sb[:, c0:c0 + w], in0=tmp2[:, :w],
                    scalar=pa_sb[:], in1=tmp2[:, :w], op0=Alu.mult,
                    op1=Alu.max)
                transpose_to_shard(h0_sb, c0, w, shard[0])
            encctx.close()

            psA = ctx.enter_context(tc.tile_pool(name="psA", bufs=2,
                                                 space="PSUM"))
            psB = ctx.enter_context(tc.tile_pool(name="psB", bufs=2,
                                                 space="PSUM"))

            nc.gpsimd.collective_compute(
                "AllGather", Alu.bypass, replica_groups=rg,
                ins=[shard[0][:, :].opt()],
                outs=[table[0][:, :].opt()])

            if debug_outputs:
                for t in range(ntile):
                    c0 = t * 512
                    w = min(512, nsh - c0)
                    dh = workp.tile([D, 512], dt.float32, tag="dbg_h")
                    nc.vector.tensor_copy(out=dh[:, :w],
                                          in_=h0_sb[:, c0:c0 + w])
                    nc.sync.dma_start(out=dbg["d_h0"][:, c0:c0 + w],
                                      in_=dh[:, :w])
                nc.sync.dma_start(out=dbg["d_table0"][:, :],
                                  in_=table[0][:, :])

            # ================= rgcn layers =================
            def emit_layer(L, h_in, table_in, h_out, shard_out):
                ws = wl_sb[L]
                for g in range(ngrp):
                    g0 = g * group
                    gw = min(group, nsh - g0)
                    aggs = [None, None]
                    for r in range(NREL):
                        ents = [e for e in sched[g][r] if e is not None]
                        if not ents:
                            continue
                        gt = gbufs[state["gb"] % NGBUF]
                        state["gb"] += 1
                        for b in range(nbank):
                            ent = sched[g][r][b]
                            if ent is None:
                                continue
                            seg_id, coff16, blk0, nblk = ent
                            num = nblk * 128
                            q = seg_id % NQUEUE
                            reg = regs[q]
                            nc.gpsimd.reg_load(
                                reg, gpad_sb[0:1, seg_id:seg_id + 1])
                            nc.gpsimd.reg_alu(reg, num, reg, Alu.subtract)
                            nc.gpsimd.dma_gather(
                                out_ap=gt[:, blk0:blk0 + nblk, :],
                                in_ap=table_in[
                                    b * bank:min((b + 1) * bank, n_nodes),
                                    :],
                                idxs_ap=gidx_sb[:, coff16:coff16 + nblk * 8],
                                num_idxs=num, num_idxs_reg=reg,
                                elem_size=D, single_packet=False,
                                queue_num=q)
                        agg = psA.tile([128, group], dt.float32, tag="agg")
                        tot = sum(e[3] for e in ents)
                        mi = 0
                        for b in range(nbank):
                            ent = sched[g][r][b]
                            if ent is None:
                                continue
                            seg_id, coff16, blk0, nblk = ent
                            cc0 = chcol[(g, r, b)]
                            for k in range(nblk):
                                S = spool.tile([128, group], dt.bfloat16,
                                               tag="S")
                                nc.vector.tensor_scalar(
                                    out=S[:], in0=iota_sb[:],
                                    scalar1=gdst_sb[:, cc0 + k:cc0 + k + 1],
                                    scalar2=gscl_sb[:, cc0 + k:cc0 + k + 1],
                                    op0=Alu.is_equal, op1=Alu.mult)
                                nc.tensor.matmul(
                                    out=agg[:], lhsT=gt[:, blk0 + k, :],
                                    rhs=S[:],
                                    start=(mi == 0), stop=(mi == tot - 1))
                                mi += 1
                        asb = workp.tile([128, group], dt.bfloat16,
                                         tag=f"agg_sb{r}")
                        nc.scalar.activation(out=asb[:], in_=agg[:],
                                             func=Act.Copy)
                        aggs[r] = asb
                    mats = [(ws["root"], h_in[:, g0:g0 + gw])]
                    if aggs[0] is not None:
                        mats.append((ws["rel0"], aggs[0][:, :gw]))
                    if aggs[1] is not None:
                        mats.append((ws["rel1"], aggs[1][:, :gw]))
                    hn = psB.tile([128, group], dt.float32, tag="hn")
                    for mi2, (wm, rhs) in enumerate(mats):
                        nc.tensor.matmul(out=hn[:, :gw], lhsT=wm[:],
                                         rhs=rhs, start=(mi2 == 0),
                                         stop=(mi2 == len(mats) - 1))
                    if h_out is not None:
                        nc.scalar.activation(out=h_out[:, g0:g0 + gw],
                                             in_=hn[:, :gw],
                                             func=Act.Identity,
                                             bias=ws["bias"][:])
                        transpose_to_shard(h_out, g0, gw, shard_out)
                    else:
                        h2t = workp.tile([128, group], dt.bfloat16,
                                         tag="h2t")
                        nc.scalar.activation(out=h2t[:, :gw],
                                             in_=hn[:, :gw],
                                             func=Act.Identity,
                                             bias=ws["bias"][:])
                        cls = psC.tile([128, group], dt.float32, tag="aux")
                        nc.tensor.matmul(out=cls[:, :gw], lhsT=wcls_sb[:],
                                         rhs=h2t[:, :gw], start=True,
                                         stop=True)
                        ot = workp.tile([128, group], dt.float32, tag="ot")
                        nc.scalar.activation(out=ot[:, :gw],
                                             in_=cls[:, :gw],
                                             func=Act.Identity,
                                             bias=bcls_sb[:])
                        nc.sync.dma_start(out=out_t[:, g0:g0 + gw],
                                          in_=ot[:, :gw])

            emit_layer(1, h0_sb, table[0], h1_sb, shard[1])
            nc.gpsimd.collective_compute(
                "AllGather", Alu.bypass, replica_groups=rg,
                ins=[shard[1][:, :].opt()],
                outs=[table[1][:, :].opt()])
            if debug_outputs:
                for t in range(ntile):
                    c0 = t * 512
                    w = min(512, nsh - c0)
                    dh = workp.tile([D, 512], dt.float32, tag="dbg_h")
                    nc.vector.tensor_copy(out=dh[:, :w],
                                          in_=h1_sb[:, c0:c0 + w])
                    nc.sync.dma_start(out=dbg["d_h1"][:, c0:c0 + w],
                                      in_=dh[:, :w])
            emit_layer(2, h1_sb, table[1], None, None)

    nc.compile()
    return nc


# ================================================================ in_maps
def make_in_maps(inputs, pack, n_nodes=N_NODES, ncores=NCORES, group=GROUP):
    x = np.asarray(inputs["x"], np.float32)
    nsh = n_nodes // ncores

    def f32(v):
        return np.ascontiguousarray(np.asarray(v, np.float32))

    def bf16(v):
        return np.ascontiguousarray(np.asarray(v, np.float32).astype(BF16))

    def col(v):
        return f32(v).reshape(-1, 1)

    iota = np.broadcast_to(np.arange(group, dtype=np.float32), (128, group))
    ident = np.eye(128, dtype=np.float32)

    common = dict(
        w_num=f32(inputs["W_num"]), w_cat=f32(inputs["W_cat"]),
        w_in=f32(inputs["W_in"]),
        b_num=col(inputs["b_num"]), b_cat=col(inputs["b_cat"]),
        b_in=col(inputs["b_in"]), prelu_a=col(inputs["prelu_a"]),
        w_root1=bf16(inputs["W_root1"]),
        w_rel10=bf16(np.asarray(inputs["W_rel1"])[0]),
        w_rel11=bf16(np.asarray(inputs["W_rel1"])[1]),
        b_rgcn1=col(inputs["b_rgcn1"]),
        w_root2=bf16(inputs["W_root2"]),
        w_rel20=bf16(np.asarray(inputs["W_rel2"])[0]),
        w_rel21=bf16(np.asarray(inputs["W_rel2"])[1]),
        b_rgcn2=col(inputs["b_rgcn2"]),
        w_cls=bf16(inputs["W_cls"]), b_cls=col(inputs["b_cls"]),
        iotaG=bf16(iota), ident128=bf16(ident),
    )
    maps = []
    for c in range(ncores):
        xs = x[c * nsh:(c + 1) * nsh]
        m = dict(common)
        m["xnum"] = np.ascontiguousarray(xs[:, :NUM_PROP].T)
        m["xcat"] = np.ascontiguousarray(xs[:, NUM_PROP:].T)
        m["gidx"] = np.ascontiguousarray(np.tile(pack["gidx"][c], (8, 1)))
        m["gdst"] = np.ascontiguousarray(pack["gdst"][c])
        m["gscl"] = np.ascontiguousarray(pack["gscl"][c])
        m["gpad"] = np.ascontiguousarray(pack["gpad"][c][None, :])
        maps.append(m)
    return maps


# ================================================================== runner
def kernel(**inputs) -> np.ndarray:
    _ensure_path()
    from concourse.bass_utils import run_bass_kernel_spmd

    edge_index = np.asarray(inputs["edge_index"])
    edge_type = np.asarray(inputs["edge_type"])
    pack = preprocess(edge_index, edge_type)
    nc = build_program(pack)
    in_maps = make_in_maps(inputs, pack)
    trace = bool(int(os.environ.get("KERNEL_TRACE", "0")))
    res = run_bass_kernel_spmd(nc, in_maps, core_ids=list(range(NCORES)),
                               trace=trace)
    if trace:
        kernel.last_results = res
    nsh = N_NODES // NCORES
    out = np.empty((N_NODES, D), np.float32)
    for c in range(NCORES):
        out[c * nsh:(c + 1) * nsh] = res.results[c]["out_t"].T
    return out


kernel.last_results = None
